# revision 1
# baseline (speedup 1.0000x reference)
"""DKVMN write-head memory update kernel for Trainium2 (8 NeuronCores).

Computes, for each batch row b:
    erase = sigmoid(control @ erase_W.T + erase_b)          # [B, D]
    add   = tanh(control @ add_W.T + add_b)                 # [B, D]
    new_memory[b,m,d] = memory[b,m,d] * (1 - ww[b,m]*erase[b,d]) + ww[b,m]*add[b,d]

Sharding: pure data parallel over batch B (4096 -> 512 per core), with the
tiny 128x128 Linear weights replicated on every core.  The host pre-transposes
control_input and the two Linear weights (cheap, 256 KB total) so the device
needs no PE transposes at all — every PE matmul then depends only on
DVE-staged operands, keeping it at a single sync-wait (the walrus fp32
LDWEIGHTS path rejects matmuls with more than one).

Per-core layout: partition dim = batch (tiles of 128 b's), free dim = (m, d)
chunks of the [128, 128] per-b memory matrix.  Rewriting the update as

    z   = add[b,:] - erase[b,:] * memory[b,m,:]      (broadcast over m)
    out = memory[b,m,:] + ww[b,m] * z

gives two full-size chunked elementwise passes (v = mem*e on DVE, z = a - v
on GPSIMD) plus a per-m fused scalar_tensor_tensor (out = (z * w_m) + mem)
whose per-partition scalar is ww[:, m], split between DVE and GPSIMD.  Each
engine stays under the ~187us/core HBM roofline (67 MB of traffic per core).
"""

import sys

for _p in ("/opt/trn_rl_repo",):
    if _p not in sys.path:
        sys.path.insert(0, _p)

from contextlib import ExitStack

import numpy as np

import concourse.bass as bass
import concourse.tile as tile
from concourse import mybir

N_CORES = 8
B, M, D = 4096, 128, 128
B_LOC = B // N_CORES  # 512
P = 128               # SBUF partitions = batch tile
N_BTILES = B_LOC // P  # 4
CHUNK_M = 16          # m-slots per chunk -> [128, 16*128] fp32 = 1 MiB tiles
N_CHUNKS = M // CHUNK_M

F32 = mybir.dt.float32
ALU = mybir.AluOpType
ACTF = mybir.ActivationFunctionType

# engine split for the per-m fused (z*w + mem) ops: every GPS_STT_EVERY-th
# m-slot goes to GPSIMD, the rest to DVE. 0 disables GPSIMD for these
# (walrus rejects TensorScalarPtr on the Pool engine, so keep this 0).
GPS_STT_EVERY = 0
BUFS_MEM = 5
BUFS_WORK = 4
BUFS_OUT = 4


def legalize_waits(nc: bass.Bass) -> None:
    """Split multi-wait instructions for walrus.

    TRN2 codegen ('setupSyncWait: Too many sync wait commands') rejects
    instructions carrying more than one semaphore wait, but the Tile
    scheduler freely attaches several (including on Rust-emitted loop
    back-edge drains).  Hoist all but the last wait onto standalone
    single-wait sequencer instructions (InstEventSemaphore — the same
    thing raw bass wait_ge emits) inserted immediately before the
    instruction on the same engine."""
    for bb in nc.main_func.blocks:
        insts = bb.instructions
        if not any(
            i.sync_info is not None and i.sync_info.on_wait and len(i.sync_info.on_wait) > 1
            for i in insts
        ):
            continue
        new_list = []
        for inst in insts:
            si = inst.sync_info
            if si is not None and si.on_wait and len(si.on_wait) > 1:
                for w in si.on_wait[:-1]:
                    ev = mybir.InstEventSemaphore(
                        name=nc.get_next_instruction_name(),
                        engine=inst.engine,
                        ins=[],
                        outs=[],
                        sync_info=mybir.SyncInfo(on_wait=[w], on_update=[]),
                    )
                    nc.register_instruction(ev, overwrite=True)
                    new_list.append(ev)
                inst.sync_info = mybir.SyncInfo(
                    on_wait=[si.on_wait[-1]], on_update=list(si.on_update)
                )
            new_list.append(inst)
        bb.instructions = new_list


def build_nc(repeat: int = 1, mode: str = "full", loop: int = 0) -> bass.Bass:
    """mode: 'full' (real kernel), 'dma' (loads+stores only), 'vz' (no per-m
    STT pass).  'dma' and 'vz' produce WRONG output — timing bisection only.
    loop > 0 wraps the whole body in a hardware For_i loop executing it that
    many times (timing only — output identical since addresses are static)."""
    nc = bass.Bass()

    # host-transposed inputs: ctrl_t[k, b] = control[b, k]; *_w_t[k, j] = W[j, k]
    ctrlT_d = nc.dram_tensor("ctrl_t", [D, B_LOC], F32, kind="ExternalInput")
    mem_d = nc.dram_tensor("memory", [B_LOC, M, D], F32, kind="ExternalInput")
    ww_d = nc.dram_tensor("write_weight", [B_LOC, M], F32, kind="ExternalInput")
    ewT_d = nc.dram_tensor("erase_w_t", [D, D], F32, kind="ExternalInput")
    eb_d = nc.dram_tensor("erase_b", [D], F32, kind="ExternalInput")
    awT_d = nc.dram_tensor("add_w_t", [D, D], F32, kind="ExternalInput")
    ab_d = nc.dram_tensor("add_b", [D], F32, kind="ExternalInput")
    out_d = nc.dram_tensor("new_memory", [B_LOC, M, D], F32, kind="ExternalOutput")

    with tile.TileContext(nc) as tc, ExitStack() as ctx:
        singles = ctx.enter_context(tc.tile_pool(name="singles", bufs=1))
        sig = ctx.enter_context(tc.tile_pool(name="sig", bufs=2))
        big = ctx.enter_context(tc.tile_pool(name="big", bufs=BUFS_MEM))
        work = ctx.enter_context(tc.tile_pool(name="work", bufs=BUFS_WORK))
        outp = ctx.enter_context(tc.tile_pool(name="outp", bufs=BUFS_OUT))
        psum = ctx.enter_context(tc.tile_pool(name="psum", bufs=1, space="PSUM"))

        # ---- one-time constants ----
        # Linear weights, DVE-staged so the signal matmuls' waits all funnel
        # through the DVE semaphore (walrus allows only one wait on fp32 LDW).
        wt_tiles = {}
        for name, w_dram in (("e", ewT_d), ("a", awT_d)):
            w_raw = sig.tile([D, D], F32, tag="wload")
            nc.sync.dma_start(out=w_raw[:], in_=w_dram[:, :])
            w_t = singles.tile([D, D], F32, tag=f"wt_{name}")
            nc.vector.tensor_copy(w_t[:], w_raw[:])
            wt_tiles[name] = w_t

        # biases replicated across partitions via partition-broadcast DMA
        bias_bc = {}
        for name, b_dram in (("e", eb_d), ("a", ab_d)):
            b_t = singles.tile([P, D], F32, tag=f"bias_{name}")
            b_ap = bass.AP(tensor=b_dram[:].tensor, offset=0, ap=[[0, P], [1, D]])
            nc.sync.dma_start(out=b_t[:], in_=b_ap)
            bias_bc[name] = b_t

        # ---- main loops (software-pipelined emission) ----
        # Each chunk's tail (per-m STT + store) is emitted AFTER the next
        # chunk's head (load + v + z), so the static per-engine instruction
        # order lets DVE start chunk i+1's v while GPSIMD still runs chunk
        # i's z — otherwise DVE stalls on the cross-engine dependency.
        def emit_tail(v_t, mem_t, w_sb, b0, m0):
            if mode == "vz":
                # timing bisection: store z directly, skip the per-m STT pass
                nc.scalar.dma_start(
                    out=out_d[b0 : b0 + P, m0 : m0 + CHUNK_M, :], in_=v_t[:]
                )
                return
            # out[:, m, :] = (z[:, m, :] * ww[:, m]) + mem[:, m, :]
            out_t = outp.tile([P, CHUNK_M, D], F32, tag="out")
            for m in range(CHUNK_M):
                mm = m0 + m
                eng = (
                    nc.gpsimd
                    if GPS_STT_EVERY and (mm % GPS_STT_EVERY == GPS_STT_EVERY - 1)
                    else nc.vector
                )
                eng.scalar_tensor_tensor(
                    out=out_t[:, m, :],
                    in0=v_t[:, m, :],
                    scalar=w_sb[:, mm : mm + 1],
                    in1=mem_t[:, m, :],
                    op0=ALU.mult,
                    op1=ALU.add,
                )
            nc.scalar.dma_start(
                out=out_d[b0 : b0 + P, m0 : m0 + CHUNK_M, :], in_=out_t[:]
            )

        def emit_body():
            pending = None
            for _rep, bt in ((r, t) for r in range(repeat) for t in range(N_BTILES)):
                b0 = bt * P

                # ctrl^T tile for this batch tile, DVE-staged (see above)
                ctrlT_raw = sig.tile([D, P], F32, tag="ctrl_raw")
                nc.sync.dma_start(out=ctrlT_raw[:], in_=ctrlT_d[:, b0 : b0 + P])
                ctrlT_sb = sig.tile([D, P], F32, tag="ctrl_stg")
                nc.vector.tensor_copy(ctrlT_sb[:], ctrlT_raw[:])

                # erase / add signals: psum[b, j] = sum_k ctrl[b,k] W[j,k],
                # then DVE adds the (partition-broadcast) bias, ACT applies the
                # nonlinearity.
                e_sb = sig.tile([P, D], F32, tag="e")
                a_sb = sig.tile([P, D], F32, tag="a")
                for name, act_fn, dst in (("e", ACTF.Sigmoid, e_sb), ("a", ACTF.Tanh, a_sb)):
                    sig_ps = psum.tile([P, D], F32, tag=f"sig_{name}")
                    nc.tensor.matmul(sig_ps[:], ctrlT_sb[:], wt_tiles[name][:])
                    pre_sb = sig.tile([P, D], F32, tag=f"pre_{name}")
                    nc.vector.tensor_tensor(pre_sb[:], sig_ps[:], bias_bc[name][:], ALU.add)
                    nc.scalar.activation(dst[:], pre_sb[:], act_fn)

                w_sb = sig.tile([P, M], F32, tag="w")
                nc.sync.dma_start(out=w_sb[:], in_=ww_d[b0 : b0 + P, :])

                for ci in range(N_CHUNKS):
                    m0 = ci * CHUNK_M

                    mem_t = big.tile([P, CHUNK_M, D], F32, tag="mem")
                    nc.sync.dma_start(
                        out=mem_t[:], in_=mem_d[b0 : b0 + P, m0 : m0 + CHUNK_M, :]
                    )

                    if mode == "dma":
                        # timing bisection: store the loaded tile straight back
                        nc.scalar.dma_start(
                            out=out_d[b0 : b0 + P, m0 : m0 + CHUNK_M, :], in_=mem_t[:]
                        )
                        continue

                    e_bc = e_sb[:].unsqueeze(1).broadcast_to((P, CHUNK_M, D))
                    a_bc = a_sb[:].unsqueeze(1).broadcast_to((P, CHUNK_M, D))

                    # v = mem * erase  (DVE), then z = add - v in place (GPSIMD)
                    v_t = work.tile([P, CHUNK_M, D], F32, tag="v")
                    nc.vector.tensor_tensor(v_t[:], mem_t[:], e_bc, ALU.mult)
                    nc.gpsimd.tensor_tensor(v_t[:], a_bc, v_t[:], ALU.subtract)

                    if pending is not None:
                        emit_tail(*pending)
                    pending = (v_t, mem_t, w_sb, b0, m0)

            if pending is not None:
                emit_tail(*pending)

        if loop:
            with tc.For_i(0, loop, 1, hint_engines=(mybir.EngineType.DVE,)):
                emit_body()
        else:
            emit_body()

    legalize_waits(nc)
    return nc


_CACHE: dict = {}


def _get_nc() -> bass.Bass:
    if "nc" not in _CACHE:
        _CACHE["nc"] = build_nc()
    return _CACHE["nc"]


def make_in_maps(**inputs) -> list:
    """Shard full inputs into per-core input maps (batch split, weights
    replicated).  control_input and the Linear weights are pre-transposed on
    the host so the device kernel needs no PE transposes."""
    ci = np.asarray(inputs["control_input"], dtype=np.float32)
    mem = np.asarray(inputs["memory"], dtype=np.float32)
    ww = np.asarray(inputs["write_weight"], dtype=np.float32)
    ewT = np.ascontiguousarray(np.asarray(inputs["erase_W"], dtype=np.float32).T)
    eb = np.ascontiguousarray(np.asarray(inputs["erase_b"], dtype=np.float32))
    awT = np.ascontiguousarray(np.asarray(inputs["add_W"], dtype=np.float32).T)
    ab = np.ascontiguousarray(np.asarray(inputs["add_b"], dtype=np.float32))
    in_maps = []
    for c in range(N_CORES):
        sl = slice(c * B_LOC, (c + 1) * B_LOC)
        in_maps.append(
            {
                "ctrl_t": np.ascontiguousarray(ci[sl].T),
                "memory": np.ascontiguousarray(mem[sl]),
                "write_weight": np.ascontiguousarray(ww[sl]),
                "erase_w_t": ewT,
                "erase_b": eb,
                "add_w_t": awT,
                "add_b": ab,
            }
        )
    return in_maps


def run_sharded(trace: bool = False, **inputs):
    """Run on all 8 cores; returns (full_output, BassKernelResults)."""
    from concourse.bass_utils import run_bass_kernel_spmd

    nc = _get_nc()
    res = run_bass_kernel_spmd(
        nc, make_in_maps(**inputs), core_ids=list(range(N_CORES)), trace=trace
    )
    out = np.concatenate(
        [res.results[c]["new_memory"] for c in range(N_CORES)], axis=0
    )
    return out, res


def kernel(**inputs) -> np.ndarray:
    out, _ = run_sharded(trace=False, **inputs)
    return out

